# revision 1
# baseline (speedup 1.0000x reference)
"""Biaffine edge attention on 8 Trainium2 NeuronCores.

Math (per batch b):
    out[i,o] = head[i,:] @ U @ dep[o,:] + head[i,:]@wh + dep[o,:]@wd + b
with head/dep [S=2048, D=256], U [D,D], edge_W = [wh | wd] (each [D]).

Sharding: pure data-parallel over batch B=8 -> one batch per core,
U / edge_W / edge_b replicated. No collectives.

Per-core kernel:
    ATf[e,i] = sum_d U[d,e] * headT[d,i] + wd[e]      (the dep-side rank-1
               term ds[o] rides the e-contraction for free)
    hs[i]    = sum_d head[i,d] * wh[d]  + b           (DVE mul+reduce;
               per-partition bias in the epilogue)
    out[i,o] = sum_e ATf[e,i] * depT[e,o]  + hs[i]
head and dep are transposed on-chip with PE transposes (batched into
[128,512] PSUM collect tiles). Matmuls run as float32r (1 cycle/row for
moving dim >= 256 vs 4 for strict fp32 => this is what makes the problem
memory- instead of compute-bound). FP32r matmul inputs must be rounded to
f32r by a compute op, so matmul-feeding SBUF tiles are float32r-typed and
written by DVE/ACT copies, never directly by DMA.

DMA sizing: inputs load as [128,1024] group tiles (4 row-blocks per DMA via
a 3D access pattern), outputs store as [128,1024] tiles -- keeps the SP
sequencer's per-DMA dispatch cost (~0.65us) well below the ~60us of data
movement.
"""

import contextlib

import numpy as np

import concourse.bass as bass
import concourse.tile as tile
from concourse import bacc, mybir
from concourse.bass_utils import run_bass_kernel_spmd

B, S, D = 8, 2048, 256
P = 128          # partitions
OC = 512         # matmul output free-dim chunk (one PSUM bank of fp32)
GB = 4           # row-blocks per input load group
NG = S // (P * GB)   # 4 load groups per input
NI = S // P      # 16 row blocks
NO = S // OC     # 4 output column chunks
ND = D // P      # 2 contraction chunks
F32 = mybir.dt.float32
F32R = mybir.dt.float32r


def build_nc(reps=1):
    """reps>1 wraps the body in a HW For_i loop -- used only for timing."""
    nc = bacc.Bacc("TRN2", target_bir_lowering=False, debug=False, num_devices=B)

    head_d = nc.dram_tensor("head", [S, D], F32, kind="ExternalInput")
    dep_d = nc.dram_tensor("dep", [S, D], F32, kind="ExternalInput")
    u_d = nc.dram_tensor("U", [D, D], F32, kind="ExternalInput")
    whr_d = nc.dram_tensor("wh_rep", [P, GB * D], F32, kind="ExternalInput")
    wdT_d = nc.dram_tensor("wdT", [P, ND], F32, kind="ExternalInput")
    b128_d = nc.dram_tensor("b128", [P, 1], F32, kind="ExternalInput")
    eye_d = nc.dram_tensor("eye", [P, P], F32, kind="ExternalInput")
    out_d = nc.dram_tensor("out", [S, S], F32, kind="ExternalOutput")

    Ident = mybir.ActivationFunctionType.Identity

    with tile.TileContext(nc) as tc:
        with (
            tc.tile_pool(name="const", bufs=1) as cpool,
            tc.tile_pool(name="persist", bufs=1) as ppool,
            tc.tile_pool(name="stage", bufs=3) as stage,
            tc.tile_pool(name="ttrp", bufs=2) as ttrp,
            tc.tile_pool(name="outbuf", bufs=3) as outbuf,
            tc.tile_pool(name="ps_t", bufs=2, space=bass.MemorySpace.PSUM) as ps_t,
            tc.tile_pool(name="ps_mm", bufs=6, space=bass.MemorySpace.PSUM) as ps_mm,
        ):
            # ---- constants ----
            eye = cpool.tile([P, P], F32, name="eye", tag="eye")
            nc.sync.dma_start(eye[:], eye_d[:])
            b128 = cpool.tile([P, 1], F32, name="b128", tag="b128")
            nc.sync.dma_start(b128[:], b128_d[:])
            wh_rep = cpool.tile([P, GB * D], F32, name="wh_rep", tag="wh_rep")
            nc.sync.dma_start(wh_rep[:], whr_d[:])
            wdT = cpool.tile([P, ND], F32, name="wdT", tag="wdT")
            nc.sync.dma_start(wdT[:], wdT_d[:])
            u_sb = []
            for dc in range(ND):
                u_stg = cpool.tile([P, D], F32, name=f"ustg{dc}", tag=f"ustg{dc}")
                nc.sync.dma_start(u_stg[:], u_d[dc * P:(dc + 1) * P, :])
                u_t = cpool.tile([P, D], F32R, name=f"u{dc}", tag=f"u{dc}")
                nc.vector.tensor_copy(u_t[:], u_stg[:])
                u_sb.append(u_t)

            # ---- persistent SBUF tensors ----
            headT = [ppool.tile([P, S], F32R, name=f"headT{dc}", tag=f"headT{dc}")
                     for dc in range(ND)]
            depT = [ppool.tile([P, S], F32R, name=f"depT{dc}", tag=f"depT{dc}")
                    for dc in range(ND)]
            atf = [ppool.tile([P, S], F32R, name=f"atf{eb}", tag=f"atf{eb}")
                   for eb in range(ND)]
            hs_col = ppool.tile([P, NI], F32, name="hs_col", tag="hs_col")
            hs_colb = ppool.tile([P, NI], F32, name="hs_colb", tag="hs_colb")

            def load_group(src_dram, g):
                # [128, GB*D]: free = (block j, d); one DMA, 3D src pattern
                nat = stage.tile([P, GB * D], F32, name="nat", tag="nat")
                src = src_dram[g * GB * P:(g + 1) * GB * P, :]
                src3 = src.rearrange("(j p) d -> p j d", p=P)
                nc.sync.dma_start(nat[:].rearrange("p (j d) -> p j d", d=D), src3)
                return nat

            def transpose_group(nat, dstT, g, eng_off):
                # 8 PE transposes -> two [128,512] PSUM collect tiles -> 2 copies
                for dc in range(ND):
                    pst = ps_t.tile([P, GB * P], F32, name="pst", tag="pst")
                    for j in range(GB):
                        nc.tensor.transpose(
                            pst[:, j * P:(j + 1) * P],
                            nat[:, j * D + dc * P: j * D + dc * P + P],
                            eye[:],
                        )
                    dst = dstT[dc][:, g * GB * P:(g + 1) * GB * P]
                    if (g * ND + dc + eng_off) % 2 == 0:
                        nc.vector.tensor_copy(dst, pst[:])
                    else:
                        nc.scalar.copy(dst, pst[:])

            def body():
                # ---- interleaved loads / transposes / hs / AT ----
                for g in range(NG):
                    nat_h = load_group(head_d, g)
                    nat_p = load_group(dep_d, g)
                    transpose_group(nat_h, headT, g, 0)
                    # hs for this group's 4 blocks: mul + blockwise reduce
                    ttr = ttrp.tile([P, GB * D], F32, name="ttr", tag="ttr")
                    nc.vector.tensor_mul(ttr[:], nat_h[:], wh_rep[:])
                    nc.vector.reduce_sum(
                        hs_col[:, g * GB:(g + 1) * GB],
                        ttr[:].rearrange("p (j d) -> p j d", d=D),
                        axis=mybir.AxisListType.X,
                    )
                    nc.scalar.activation(
                        hs_colb[:, g * GB:(g + 1) * GB],
                        hs_col[:, g * GB:(g + 1) * GB], Ident,
                        bias=b128[:, 0:1],
                    )
                    transpose_group(nat_p, depT, g, 1)
                    # ATf chunk ic=g (headT[:, g*512:(g+1)*512] just written)
                    for eb in range(ND):
                        pa = ps_mm.tile([P, OC], F32, name="psmm", tag="psmm")
                        for dc in range(ND):
                            nc.tensor.matmul(
                                pa[:],
                                u_sb[dc][:, eb * P:(eb + 1) * P],
                                headT[dc][:, g * OC:(g + 1) * OC],
                                start=(dc == 0),
                                stop=(dc == ND - 1),
                            )
                        nc.scalar.activation(
                            atf[eb][:, g * OC:(g + 1) * OC], pa[:], Ident,
                            bias=wdT[:, eb:eb + 1],
                        )

                # ---- big matmul + fused epilogue, full-row out tiles ----
                for ib in range(NI):
                    ot = outbuf.tile([P, S], F32, name="ot", tag="ot")
                    for oc in range(NO):
                        po = ps_mm.tile([P, OC], F32, name="psmm", tag="psmm")
                        for eb in range(ND):
                            nc.tensor.matmul(
                                po[:],
                                atf[eb][:, ib * P:(ib + 1) * P],
                                depT[eb][:, oc * OC:(oc + 1) * OC],
                                start=(eb == 0),
                                stop=(eb == ND - 1),
                            )
                        dst = ot[:, oc * OC:(oc + 1) * OC]
                        if (ib + oc) % 2 == 0:
                            nc.scalar.activation(
                                dst, po[:], Ident, bias=hs_colb[:, ib:ib + 1]
                            )
                        else:
                            nc.vector.tensor_scalar_add(
                                dst, po[:], hs_colb[:, ib:ib + 1]
                            )
                    nc.sync.dma_start(out_d[ib * P:(ib + 1) * P, :], ot[:])

            if reps > 1:
                with tc.For_i(0, reps, 1):
                    body()
            else:
                body()

    nc.finalize()
    return nc


_NC_CACHE = {}


def _get_nc(reps=1):
    if reps not in _NC_CACHE:
        _NC_CACHE[reps] = build_nc(reps)
    return _NC_CACHE[reps]


def make_in_maps(head, dep, edge_U, edge_W, edge_b):
    head = np.ascontiguousarray(np.asarray(head, dtype=np.float32))
    dep = np.ascontiguousarray(np.asarray(dep, dtype=np.float32))
    u = np.ascontiguousarray(np.asarray(edge_U, dtype=np.float32))
    w = np.asarray(edge_W, dtype=np.float32).reshape(-1)
    wh, wd = w[:D], w[D:]
    wh_rep = np.ascontiguousarray(np.tile(wh[None, :], (P, GB)))
    wdT = np.ascontiguousarray(wd.reshape(ND, P).T)
    b128 = np.full((P, 1), float(np.asarray(edge_b).reshape(-1)[0]), np.float32)
    eye = np.eye(P, dtype=np.float32)
    return [
        {
            "head": head[b], "dep": dep[b], "U": u,
            "wh_rep": wh_rep, "wdT": wdT, "b128": b128, "eye": eye,
        }
        for b in range(B)
    ]


def kernel(head, dep, edge_U, edge_W, edge_b):
    nc = _get_nc()
    in_maps = make_in_maps(head, dep, edge_U, edge_W, edge_b)
    res = run_bass_kernel_spmd(nc, in_maps, core_ids=list(range(B)))
    return np.stack([res.results[b]["out"] for b in range(B)], axis=0)



# revision 23
# speedup vs baseline: 1.0757x; 1.0757x over previous
"""Biaffine edge attention on 8 Trainium2 NeuronCores.

Math (per batch b):
    out[i,o] = head[i,:] @ U @ dep[o,:] + head[i,:]@wh + dep[o,:]@wd + b
with head/dep [S=2048, D=256], U [D,D], edge_W = [wh | wd] (each [D]).

Sharding: pure data-parallel over batch B=8 -> one batch per core,
constants replicated. No collectives.

Per-core kernel:
    ATf[e,i] = sum_d U[d,e] * headT[d,i] + wd[e]      (the dep-side rank-1
               term ds[o] rides the e-contraction for free)
    hs[i]    = sum_d head[i,d] * wh[d]  + b           (DVE mul+reduce;
               per-partition bias in the epilogue)
    out[i,o] = sum_e ATf[e,i] * depT[e,o]  + hs[i]
head and dep are transposed on-chip with PE transposes (batched into
[128,512] PSUM collect tiles). Matmuls run as float32r (1 cycle/row for
moving dim >= 256 => memory- instead of compute-bound). FP32r matmul
inputs are rounded to f32r by DVE/ACT copies, never written directly by
DMA.

Schedule (the whole point -- keep the DMA pipe busy end to end):
  - ALL constants ride in one packed [128,904] DMA (eye | U | wh | wdT | b)
    instead of 6 small dispatch-serialized DMAs.
  - Input loads are issued up front in the order head0, dep0..3, head1..3
    ([128,1024] group tiles, 4 row-blocks per DMA). deps come early because
    the first out row needs ALL of depT but only ATf chunk 0.
  - Transposes/ATf for head groups 1-3 are deferred into the store phase
    (PE has ~1.2us/row of slack there), so the first store issues ~13us in
    and the 16 x 1MB store train (46.6us, the true roofline term) runs
    without a bubble.
"""

import numpy as np

import concourse.bass as bass
import concourse.tile as tile
from concourse import bacc, mybir
from concourse.bass_utils import run_bass_kernel_spmd

B, S, D = 8, 2048, 256
P = 128          # partitions
OC = 512         # matmul output free-dim chunk (one PSUM bank of fp32)
GB = 4           # row-blocks per input load group
NG = S // (P * GB)   # 4 load groups per input
NI = S // P      # 16 row blocks
NO = S // OC     # 4 output column chunks
ND = D // P      # 2 contraction chunks
F32 = mybir.dt.float32
F32R = mybir.dt.float32r

# pack layout (free-dim col offsets in the [P, PACKW] constant tensor)
EYE_O = 0                # [P, P] identity
U_O = EYE_O + P          # [P, ND*D]: U rows dc*128.. as two [128,256] blocks
WH_O = U_O + ND * D      # [P, D] wh replicated across partitions
WDT_O = WH_O + D         # [P, ND] wdT[p, dc] = wd[dc*128+p]
B_O = WDT_O + ND         # [P, 1] bias replicated
PACKW = B_O + 1 + 3      # pad to 904


def build_nc(reps=1):
    """reps>1 wraps the body in a HW For_i loop -- used only for timing."""
    nc = bacc.Bacc("TRN2", target_bir_lowering=False, debug=False, num_devices=B)

    head_d = nc.dram_tensor("head", [S, D], F32, kind="ExternalInput")
    dep_d = nc.dram_tensor("dep", [S, D], F32, kind="ExternalInput")
    pack_d = nc.dram_tensor("pack", [P, PACKW], F32, kind="ExternalInput")
    out_d = nc.dram_tensor("out", [S, S], F32, kind="ExternalOutput")

    Ident = mybir.ActivationFunctionType.Identity

    with tile.TileContext(nc) as tc:
        with (
            tc.tile_pool(name="const", bufs=1) as cpool,
            tc.tile_pool(name="persist", bufs=1) as ppool,
            tc.tile_pool(name="stage", bufs=10) as stage,
            tc.tile_pool(name="ttrp", bufs=2) as ttrp,
            tc.tile_pool(name="outbuf", bufs=4) as outbuf,
            tc.tile_pool(name="ps_t", bufs=3, space=bass.MemorySpace.PSUM) as ps_t,
            tc.tile_pool(name="ps_mm", bufs=5, space=bass.MemorySpace.PSUM) as ps_mm,
        ):
            # ---- PE p-state warmup: ~3.4us of dummy f32r matmuls so the
            # tensor engine reaches full clock before the real transposes
            # (cost model: 4x slower until 3us of continuous execution) ----
            warm = cpool.tile([P, OC], F32, name="warm", tag="warm")
            nc.vector.memset(warm[:], 1.0)
            pw = ps_mm.tile([P, OC], F32, name="psmm", tag="psmm")
            for _ in range(4):
                nc.tensor.matmul(pw[:], warm[:, 0:P], warm[:],
                                 start=True, stop=True)

            # ---- constants: ONE packed DMA, first in the sync queue ----
            pack = cpool.tile([P, PACKW], F32, name="pack", tag="pack")
            nc.sync.dma_start(pack[:], pack_d[:])
            eye = pack[:, EYE_O:EYE_O + P]
            whr = pack[:, WH_O:WH_O + D]
            wdT = pack[:, WDT_O:WDT_O + ND]
            b128 = pack[:, B_O:B_O + 1]
            u_sb = []
            for dc in range(ND):
                u_t = cpool.tile([P, D], F32R, name=f"u{dc}", tag=f"u{dc}")
                nc.vector.tensor_copy(u_t[:], pack[:, U_O + dc * D:U_O + (dc + 1) * D])
                u_sb.append(u_t)

            # ---- persistent SBUF tensors ----
            headT = [ppool.tile([P, S], F32R, name=f"headT{dc}", tag=f"headT{dc}")
                     for dc in range(ND)]
            depT = [ppool.tile([P, S], F32R, name=f"depT{dc}", tag=f"depT{dc}")
                    for dc in range(ND)]
            atf = [ppool.tile([P, S], F32R, name=f"atf{eb}", tag=f"atf{eb}")
                   for eb in range(ND)]
            hs_colb = ppool.tile([P, NI], F32, name="hs_colb", tag="hs_colb")

            eng_ctr = [0]

            def load_group(src_dram, g):
                # [128, GB*D]: free = (block j, d); one DMA, 3D src pattern
                nat = stage.tile([P, GB * D], F32, name="nat", tag="nat")
                src = src_dram[g * GB * P:(g + 1) * GB * P, :]
                src3 = src.rearrange("(j p) d -> p j d", p=P)
                nc.sync.dma_start(nat[:].rearrange("p (j d) -> p j d", d=D), src3)
                return nat

            def transpose_group(nat, dstT, g):
                # 8 PE transposes -> two [128,512] PSUM collect tiles -> 2 copies
                for dc in range(ND):
                    pst = ps_t.tile([P, GB * P], F32, name="pst", tag="pst")
                    for j in range(GB):
                        nc.tensor.transpose(
                            pst[:, j * P:(j + 1) * P],
                            nat[:, j * D + dc * P: j * D + dc * P + P],
                            eye,
                        )
                    dst = dstT[dc][:, g * GB * P:(g + 1) * GB * P]
                    eng_ctr[0] += 1
                    if eng_ctr[0] % 2 == 0:
                        nc.vector.tensor_copy(dst, pst[:])
                    else:
                        nc.scalar.copy(dst, pst[:])

            def hs_group(nat, g):
                # hs for this group's 4 blocks: 4 muls + blockwise reduce + bias
                ttr = ttrp.tile([P, GB * D], F32, name="ttr", tag="ttr")
                for j in range(GB):
                    nc.vector.tensor_mul(
                        ttr[:, j * D:(j + 1) * D], nat[:, j * D:(j + 1) * D], whr
                    )
                hs_raw = ttrp.tile([P, GB], F32, name="hsr", tag="hsr")
                nc.vector.reduce_sum(
                    hs_raw[:],
                    ttr[:].rearrange("p (j d) -> p j d", d=D),
                    axis=mybir.AxisListType.X,
                )
                nc.scalar.activation(
                    hs_colb[:, g * GB:(g + 1) * GB], hs_raw[:], Ident, bias=b128
                )

            def atf_group(g):
                # ATf chunk ic=g from headT[:, g*512:(g+1)*512]
                for eb in range(ND):
                    pa = ps_mm.tile([P, OC], F32, name="psmm", tag="psmm")
                    for dc in range(ND):
                        nc.tensor.matmul(
                            pa[:],
                            u_sb[dc][:, eb * P:(eb + 1) * P],
                            headT[dc][:, g * OC:(g + 1) * OC],
                            start=(dc == 0),
                            stop=(dc == ND - 1),
                        )
                    # split the two bias/round chunks across ACT and DVE so
                    # atf-ready latency is one chunk, not two
                    if eb == 0:
                        nc.scalar.activation(
                            atf[eb][:, g * OC:(g + 1) * OC], pa[:], Ident,
                            bias=wdT[:, eb:eb + 1],
                        )
                    else:
                        nc.vector.tensor_scalar_add(
                            atf[eb][:, g * OC:(g + 1) * OC], pa[:],
                            wdT[:, eb:eb + 1],
                        )

            def out_row(ib):
                # one [128, 2048] out row: 8 matmuls + fused bias epilogue
                # row 0 computes its depT3-dependent chunk first -- that
                # chunk is the last to unblock, and starting with it lets
                # the rest pipeline behind it on the ramp
                ot = outbuf.tile([P, S], F32, name="ot", tag="ot")
                oc_order = range(NO - 1, -1, -1) if ib == 0 else range(NO)
                for oc in oc_order:
                    po = ps_mm.tile([P, OC], F32, name="psmm", tag="psmm")
                    for eb in range(ND):
                        nc.tensor.matmul(
                            po[:],
                            atf[eb][:, ib * P:(ib + 1) * P],
                            depT[eb][:, oc * OC:(oc + 1) * OC],
                            start=(eb == 0),
                            stop=(eb == ND - 1),
                        )
                    dst = ot[:, oc * OC:(oc + 1) * OC]
                    if (ib + oc) % 2 == 0:
                        nc.scalar.activation(
                            dst, po[:], Ident, bias=hs_colb[:, ib:ib + 1]
                        )
                    else:
                        nc.vector.tensor_scalar_add(
                            dst, po[:], hs_colb[:, ib:ib + 1]
                        )
                # first stores ride the (now idle, low-latency HWDGE) sync
                # queue; the rest ride gpsimd/SWDGE so next rep's loads on
                # sync prefetch during this rep's store train
                if ib < 4:
                    nc.sync.dma_start(out_d[ib * P:(ib + 1) * P, :], ot[:])
                else:
                    nc.gpsimd.dma_start(out_d[ib * P:(ib + 1) * P, :], ot[:])

            def body():
                # ---- all loads up front: head0, dep0..3, head1..3 ----
                nat_h = {0: load_group(head_d, 0)}
                nat_p = [load_group(dep_d, g) for g in range(NG)]
                for g in range(1, NG):
                    nat_h[g] = load_group(head_d, g)

                # ---- pre-phase PE work: T(h0), ATf0, T(p0..3) ----
                # (hs after atf: keeps DVE free for the PSUM collect copies
                # that gate ATf0 -> first store)
                transpose_group(nat_h[0], headT, 0)
                atf_group(0)
                hs_group(nat_h[0], 0)
                for g in range(NG):
                    transpose_group(nat_p[g], depT, g)

                # ---- store train; head groups 1-3 interleave in PE slack ----
                for ib in range(NI):
                    if ib % GB == 0 and ib > 0:
                        k = ib // GB
                        transpose_group(nat_h[k], headT, k)
                        atf_group(k)
                        hs_group(nat_h[k], k)
                    out_row(ib)

            if reps > 1:
                with tc.For_i(0, reps, 1):
                    body()
            else:
                body()

    nc.finalize()
    return nc


_NC_CACHE = {}


def _get_nc(reps=1):
    if reps not in _NC_CACHE:
        _NC_CACHE[reps] = build_nc(reps)
    return _NC_CACHE[reps]


def make_in_maps(head, dep, edge_U, edge_W, edge_b):
    head = np.ascontiguousarray(np.asarray(head, dtype=np.float32))
    dep = np.ascontiguousarray(np.asarray(dep, dtype=np.float32))
    u = np.asarray(edge_U, dtype=np.float32)
    w = np.asarray(edge_W, dtype=np.float32).reshape(-1)
    wh, wd = w[:D], w[D:]
    pack = np.zeros((P, PACKW), np.float32)
    pack[:, EYE_O:EYE_O + P] = np.eye(P, dtype=np.float32)
    for dc in range(ND):
        pack[:, U_O + dc * D:U_O + (dc + 1) * D] = u[dc * P:(dc + 1) * P, :]
    pack[:, WH_O:WH_O + D] = wh[None, :]
    pack[:, WDT_O:WDT_O + ND] = wd.reshape(ND, P).T
    pack[:, B_O] = float(np.asarray(edge_b).reshape(-1)[0])
    return [
        {"head": head[b], "dep": dep[b], "pack": pack}
        for b in range(B)
    ]


def kernel(head, dep, edge_U, edge_W, edge_b):
    nc = _get_nc()
    in_maps = make_in_maps(head, dep, edge_U, edge_W, edge_b)
    res = run_bass_kernel_spmd(nc, in_maps, core_ids=list(range(B)))
    return np.stack([res.results[b]["out"] for b in range(B)], axis=0)


# revision 31
# speedup vs baseline: 1.1361x; 1.0562x over previous
"""Biaffine edge attention on 8 Trainium2 NeuronCores.

Math (per batch b):
    out[i,o] = head[i,:] @ U @ dep[o,:] + head[i,:]@wh + dep[o,:]@wd + b
with head/dep [S=2048, D=256], U [D,D], edge_W = [wh | wd] (each [D]).

Sharding: pure data-parallel over batch B=8 -> one batch per core,
constants replicated. No collectives.

Per-core kernel:
    ATf[e,i] = sum_d U[d,e] * headT[d,i] + wd[e]      (the dep-side rank-1
               term ds[o] rides the e-contraction for free)
    hs[i]    = sum_d head[i,d] * wh[d]  + b           (DVE mul+reduce;
               per-partition bias in the epilogue)
    out[i,o] = sum_e ATf[e,i] * depT[e,o]  + hs[i]
head and dep are transposed on-chip with PE transposes (batched into
[128,512] PSUM collect tiles). Matmuls run as float32r (1 cycle/row for
moving dim >= 256 => memory- instead of compute-bound). FP32r matmul
inputs are rounded to f32r by DVE/ACT copies, never written directly by
DMA.

Schedule (the whole point -- keep the DMA pipe busy end to end):
  - ALL constants ride in one packed [128,904] DMA (eye | U | wh | wdT | b)
    instead of 6 small dispatch-serialized DMAs.
  - Input loads are issued up front in the order head0, dep0..3, head1..3
    ([128,1024] group tiles, 4 row-blocks per DMA). deps come early because
    the first out row needs ALL of depT but only ATf chunk 0.
  - Transposes/ATf for head groups 1-3 are deferred into the store phase
    (PE has ~1.2us/row of slack there), so the first store issues ~13us in
    and the 16 x 1MB store train (46.6us, the true roofline term) runs
    without a bubble.
"""

import numpy as np

import concourse.bass as bass
import concourse.tile as tile
from concourse import bacc, mybir
from concourse.bass_utils import run_bass_kernel_spmd

B, S, D = 8, 2048, 256
P = 128          # partitions
OC = 512         # matmul output free-dim chunk (one PSUM bank of fp32)
GB = 4           # row-blocks per input load group
NG = S // (P * GB)   # 4 load groups per input
NI = S // P      # 16 row blocks
NO = S // OC     # 4 output column chunks
ND = D // P      # 2 contraction chunks
F32 = mybir.dt.float32
F32R = mybir.dt.float32r

# pack layout (free-dim col offsets in the [P, PACKW] constant tensor)
EYE_O = 0                # [P, P] identity
U_O = EYE_O + P          # [P, ND*D]: U rows dc*128.. as two [128,256] blocks
WH_O = U_O + ND * D      # [P, D] wh replicated across partitions
WDT_O = WH_O + D         # [P, ND] wdT[p, dc] = wd[dc*128+p]
B_O = WDT_O + ND         # [P, 1] bias replicated
PACKW = B_O + 1 + 3      # pad to 904


def build_nc(reps=1, unroll=False):
    """reps>1 wraps the body in a HW For_i loop -- used only for timing.
    unroll=True duplicates the body python-side instead (TimelineSim can't
    branch); only used by the local profiling harness."""
    nc = bacc.Bacc("TRN2", target_bir_lowering=False, debug=False, num_devices=B)

    head_d = nc.dram_tensor("head", [S, D], F32, kind="ExternalInput")
    dep_d = nc.dram_tensor("dep", [S, D], F32, kind="ExternalInput")
    pack_d = nc.dram_tensor("pack", [P, PACKW], F32, kind="ExternalInput")
    out_d = nc.dram_tensor("out", [S, S], F32, kind="ExternalOutput")

    Ident = mybir.ActivationFunctionType.Identity

    with tile.TileContext(nc) as tc:
        with (
            tc.tile_pool(name="const", bufs=1) as cpool,
            tc.tile_pool(name="persist", bufs=1) as ppool,
            tc.tile_pool(name="stage", bufs=10) as stage,
            tc.tile_pool(name="ttrp", bufs=2) as ttrp,
            tc.tile_pool(name="outbuf", bufs=4) as outbuf,
            tc.tile_pool(name="ps_t", bufs=3, space=bass.MemorySpace.PSUM) as ps_t,
            tc.tile_pool(name="ps_mm", bufs=5, space=bass.MemorySpace.PSUM) as ps_mm,
        ):
            # ---- PE p-state warmup: ~3.4us of dummy f32r matmuls so the
            # tensor engine reaches full clock before the real transposes
            # (cost model: 4x slower until 3us of continuous execution) ----
            warm = cpool.tile([P, OC], F32, name="warm", tag="warm")
            nc.vector.memset(warm[:], 1.0)
            pw = ps_mm.tile([P, OC], F32, name="psmm", tag="psmm")
            # one strict-f32 matmul at cold clock runs ~3.4us -- exactly the
            # continuous-busy credit the PE needs to reach full p-state
            nc.tensor.matmul(pw[:], warm[:, 0:P], warm[:], start=True, stop=True)

            # ---- constants: ONE packed DMA, first in the sync queue ----
            pack = cpool.tile([P, PACKW], F32, name="pack", tag="pack")
            nc.sync.dma_start(pack[:], pack_d[:])
            eye = pack[:, EYE_O:EYE_O + P]
            whr = pack[:, WH_O:WH_O + D]
            wdT = pack[:, WDT_O:WDT_O + ND]
            b128 = pack[:, B_O:B_O + 1]
            u_sb = []
            for dc in range(ND):
                u_t = cpool.tile([P, D], F32R, name=f"u{dc}", tag=f"u{dc}")
                nc.vector.tensor_copy(u_t[:], pack[:, U_O + dc * D:U_O + (dc + 1) * D])
                u_sb.append(u_t)

            # ---- persistent SBUF tensors ----
            headT = [ppool.tile([P, S], F32R, name=f"headT{dc}", tag=f"headT{dc}")
                     for dc in range(ND)]
            depT = [ppool.tile([P, S], F32R, name=f"depT{dc}", tag=f"depT{dc}")
                    for dc in range(ND)]
            atf = [ppool.tile([P, S], F32R, name=f"atf{eb}", tag=f"atf{eb}")
                   for eb in range(ND)]
            hs_colb = ppool.tile([P, NI], F32, name="hs_colb", tag="hs_colb")

            eng_ctr = [0]

            def load_group(src_dram, g):
                # [128, GB*D]: free = (block j, d); one DMA, 3D src pattern
                nat = stage.tile([P, GB * D], F32, name="nat", tag="nat")
                src = src_dram[g * GB * P:(g + 1) * GB * P, :]
                src3 = src.rearrange("(j p) d -> p j d", p=P)
                nc.sync.dma_start(nat[:].rearrange("p (j d) -> p j d", d=D), src3)
                return nat

            def transpose_group(nat, dstT, g):
                # 8 PE transposes -> two [128,512] PSUM collect tiles -> 2 copies
                for dc in range(ND):
                    pst = ps_t.tile([P, GB * P], F32, name="pst", tag="pst")
                    for j in range(GB):
                        nc.tensor.transpose(
                            pst[:, j * P:(j + 1) * P],
                            nat[:, j * D + dc * P: j * D + dc * P + P],
                            eye,
                        )
                    dst = dstT[dc][:, g * GB * P:(g + 1) * GB * P]
                    eng_ctr[0] += 1
                    if eng_ctr[0] % 2 == 0:
                        nc.vector.tensor_copy(dst, pst[:])
                    else:
                        nc.scalar.copy(dst, pst[:])

            def hs_group(nat, g):
                # hs for this group's 4 blocks: 4 muls + blockwise reduce + bias
                ttr = ttrp.tile([P, GB * D], F32, name="ttr", tag="ttr")
                for j in range(GB):
                    nc.vector.tensor_mul(
                        ttr[:, j * D:(j + 1) * D], nat[:, j * D:(j + 1) * D], whr
                    )
                hs_raw = ttrp.tile([P, GB], F32, name="hsr", tag="hsr")
                nc.vector.reduce_sum(
                    hs_raw[:],
                    ttr[:].rearrange("p (j d) -> p j d", d=D),
                    axis=mybir.AxisListType.X,
                )
                nc.scalar.activation(
                    hs_colb[:, g * GB:(g + 1) * GB], hs_raw[:], Ident, bias=b128
                )

            def atf_group(g):
                # ATf chunk ic=g from headT[:, g*512:(g+1)*512]
                for eb in range(ND):
                    pa = ps_mm.tile([P, OC], F32, name="psmm", tag="psmm")
                    for dc in range(ND):
                        nc.tensor.matmul(
                            pa[:],
                            u_sb[dc][:, eb * P:(eb + 1) * P],
                            headT[dc][:, g * OC:(g + 1) * OC],
                            start=(dc == 0),
                            stop=(dc == ND - 1),
                        )
                    # split the two bias/round chunks across ACT and DVE so
                    # atf-ready latency is one chunk, not two
                    if eb == 0:
                        nc.scalar.activation(
                            atf[eb][:, g * OC:(g + 1) * OC], pa[:], Ident,
                            bias=wdT[:, eb:eb + 1],
                        )
                    else:
                        nc.vector.tensor_scalar_add(
                            atf[eb][:, g * OC:(g + 1) * OC], pa[:],
                            wdT[:, eb:eb + 1],
                        )

            def out_row(ib):
                # one [128, 2048] out row: 8 matmuls + fused bias epilogue.
                # Row 0 is the ramp: its depT3-dependent chunk (oc=3) is the
                # last to unblock, so compute it first and store each chunk
                # the moment its epilogue lands -- the store train starts
                # ~0.2us after the last input load instead of ~1.8us.
                ot = outbuf.tile([P, S], F32, name="ot", tag="ot")
                oc_order = range(NO - 1, -1, -1) if ib == 0 else range(NO)
                for oc in oc_order:
                    po = ps_mm.tile([P, OC], F32, name="psmm", tag="psmm")
                    for eb in range(ND):
                        nc.tensor.matmul(
                            po[:],
                            atf[eb][:, ib * P:(ib + 1) * P],
                            depT[eb][:, oc * OC:(oc + 1) * OC],
                            start=(eb == 0),
                            stop=(eb == ND - 1),
                        )
                    dst = ot[:, oc * OC:(oc + 1) * OC]
                    if (ib + oc) % 2 == 0:
                        nc.scalar.activation(
                            dst, po[:], Ident, bias=hs_colb[:, ib:ib + 1]
                        )
                    else:
                        nc.vector.tensor_scalar_add(
                            dst, po[:], hs_colb[:, ib:ib + 1]
                        )
                    if ib == 0:
                        nc.sync.dma_start(
                            out_d[ib * P:(ib + 1) * P, oc * OC:(oc + 1) * OC],
                            dst,
                        )
                # all stores on the sync/HWDGE queue: SP is otherwise idle
                # and HWDGE latency (~0.6us) beats SWDGE (~1.7us); the For_i
                # all-engine barrier rules out cross-rep prefetch anyway
                if ib > 0:
                    nc.sync.dma_start(out_d[ib * P:(ib + 1) * P, :], ot[:])

            def body():
                # ---- all loads up front: head0, dep0..3, head1..3 ----
                nat_h = {0: load_group(head_d, 0)}
                nat_p = [load_group(dep_d, g) for g in range(NG)]
                for g in range(1, NG):
                    nat_h[g] = load_group(head_d, g)

                # ---- pre-phase PE work: T(h0), ATf0, T(p0..3) ----
                # (hs after atf: keeps DVE free for the PSUM collect copies
                # that gate ATf0 -> first store)
                transpose_group(nat_h[0], headT, 0)
                atf_group(0)
                hs_group(nat_h[0], 0)
                for g in range(NG):
                    transpose_group(nat_p[g], depT, g)

                # ---- store train; head groups 1-3 interleave in PE slack ----
                for ib in range(NI):
                    if ib % GB == 0 and ib > 0:
                        k = ib // GB
                        transpose_group(nat_h[k], headT, k)
                        atf_group(k)
                        hs_group(nat_h[k], k)
                    out_row(ib)

            nbody = int(unroll) if unroll else 1
            if reps > 1:
                with tc.For_i(0, reps, 1):
                    for _ in range(nbody):
                        body()
            else:
                for _ in range(nbody):
                    body()

    nc.finalize()
    return nc


_NC_CACHE = {}


def _get_nc(reps=1):
    if reps not in _NC_CACHE:
        _NC_CACHE[reps] = build_nc(reps)
    return _NC_CACHE[reps]


def make_in_maps(head, dep, edge_U, edge_W, edge_b):
    head = np.ascontiguousarray(np.asarray(head, dtype=np.float32))
    dep = np.ascontiguousarray(np.asarray(dep, dtype=np.float32))
    u = np.asarray(edge_U, dtype=np.float32)
    w = np.asarray(edge_W, dtype=np.float32).reshape(-1)
    wh, wd = w[:D], w[D:]
    pack = np.zeros((P, PACKW), np.float32)
    pack[:, EYE_O:EYE_O + P] = np.eye(P, dtype=np.float32)
    for dc in range(ND):
        pack[:, U_O + dc * D:U_O + (dc + 1) * D] = u[dc * P:(dc + 1) * P, :]
    pack[:, WH_O:WH_O + D] = wh[None, :]
    pack[:, WDT_O:WDT_O + ND] = wd.reshape(ND, P).T
    pack[:, B_O] = float(np.asarray(edge_b).reshape(-1)[0])
    return [
        {"head": head[b], "dep": dep[b], "pack": pack}
        for b in range(B)
    ]


def kernel(head, dep, edge_U, edge_W, edge_b):
    nc = _get_nc()
    in_maps = make_in_maps(head, dep, edge_U, edge_W, edge_b)
    res = run_bass_kernel_spmd(nc, in_maps, core_ids=list(range(B)))
    return np.stack([res.results[b]["out"] for b in range(B)], axis=0)


# revision 50
# speedup vs baseline: 1.2996x; 1.1439x over previous
"""Biaffine edge attention on 8 Trainium2 NeuronCores.

Math (per batch b):
    out[i,o] = head[i,:] @ U @ dep[o,:] + head[i,:]@wh + dep[o,:]@wd + b
with head/dep [S=2048, D=256], U [D,D], edge_W = [wh | wd] (each [D]).

Sharding: pure data-parallel over batch B=8 -> one batch per core,
constants replicated. No collectives.

Per-core kernel (all bf16 compute, f32 PSUM accumulate; harness gate is
rel_err < 2e-2 and bf16 lands ~1.5e-3):
    ATf[e,i] = sum_d U[d,e] * headT[d,i] + wd[e]      (the dep-side rank-1
               term ds[o] rides the e-contraction for free)
    hs[i]    = sum_d head[i,d] * wh[d]  + b           (gpsimd mul + DVE
               reduce; per-partition bias in the epilogue)
    out[i,o] = sum_e ATf[e,i] * depT[e,o]  + hs[i]

Key moves vs a straightforward f32 kernel:
  - Inputs are cast f32 -> bf16 *inside the load DMA* (gpsimd/SWDGE can
    cast): HBM read traffic is unchanged but on-chip everything halves,
    PE transposes run at 1 cyc/row, and the output stores in bf16 halve
    the dominant store traffic (upcast to f32 host-side).
  - Column-major sweep: out chunks [128,512] are computed col-stripe by
    col-stripe in dep-arrival order, so the store train starts ~6us in
    and the PE never waits for the last dep load. Chunk stores ride the
    sync/HWDGE queue (SP is otherwise idle).
  - Head transposes/ATf interleave with the col-0 sweep as their loads
    land; a dummy f32 matmul at t~1us absorbs the PE p-state ramp.
"""

import numpy as np

import concourse.bass as bass
import concourse.tile as tile
from concourse import bacc, mybir
from concourse.bass_utils import run_bass_kernel_spmd

B, S, D = 8, 2048, 256
P = 128          # partitions
OC = 512         # output free-dim chunk (one PSUM bank of fp32)
GB = 4           # row-blocks per input load group
NG = S // (P * GB)   # 4 load groups per input
NI = S // P      # 16 row blocks
NO = S // OC     # 4 output column chunks
ND = D // P      # 2 contraction chunks
F32 = mybir.dt.float32
BF16 = mybir.dt.bfloat16

# pack layout (free-dim col offsets in the [P, PACKW] f32 constant tensor)
EYE_O = 0                # [P, P] identity
U_O = EYE_O + P          # [P, ND*D]: U rows dc*128.. as two [128,256] blocks
WHT_O = U_O + ND * D     # [P, ND] whT[p, dc] = wh[dc*128+p]
WDT_O = WHT_O + ND       # [P, ND] wdT[p, dc] = wd[dc*128+p]
B_O = WDT_O + ND         # [P, 1] bias replicated
PACKW = B_O + 1 + 3      # pad to 648


def build_nc(reps=1, unroll=False):
    """reps>1 wraps the body in a HW For_i loop -- used only for timing.
    unroll=N duplicates the body python-side (TimelineSim can't branch)."""
    nc = bacc.Bacc("TRN2", target_bir_lowering=False, debug=False, num_devices=B)

    head_d = nc.dram_tensor("head", [S, D], F32, kind="ExternalInput")
    dep_d = nc.dram_tensor("dep", [S, D], F32, kind="ExternalInput")
    pack_d = nc.dram_tensor("pack", [P, PACKW], F32, kind="ExternalInput")
    out_d = nc.dram_tensor("out", [S, S], BF16, kind="ExternalOutput")

    Ident = mybir.ActivationFunctionType.Identity

    with tile.TileContext(nc) as tc:
        with (
            tc.tile_pool(name="const", bufs=1) as cpool,
            tc.tile_pool(name="persist", bufs=1) as ppool,
            tc.tile_pool(name="stage", bufs=10) as stage,
            tc.tile_pool(name="outbuf", bufs=6) as outbuf,
            tc.tile_pool(name="ps_t", bufs=3, space=bass.MemorySpace.PSUM) as ps_t,
            tc.tile_pool(name="ps_mm", bufs=5, space=bass.MemorySpace.PSUM) as ps_mm,
        ):
            # ---- PE p-state warmup: one strict-f32 matmul at cold clock
            # runs ~3.4us -- the continuous-busy credit the PE needs ----
            warm = cpool.tile([P, OC], F32, name="warm", tag="warm")
            nc.vector.memset(warm[:], 1.0)
            pw = ps_mm.tile([P, OC], F32, name="psmm", tag="psmm")
            nc.tensor.matmul(pw[:], warm[:, 0:P], warm[:], start=True, stop=True)

            # ---- constants: ONE packed f32 DMA on sync, bf16 copies ----
            pack = cpool.tile([P, PACKW], F32, name="pack", tag="pack")
            nc.sync.dma_start(pack[:], pack_d[:])
            wdT = pack[:, WDT_O:WDT_O + ND]          # f32 bias APs are fine
            b128 = pack[:, B_O:B_O + 1]
            eye = cpool.tile([P, P], BF16, name="eye", tag="eye")
            nc.vector.tensor_copy(eye[:], pack[:, EYE_O:EYE_O + P])
            whT = cpool.tile([P, ND], BF16, name="whT", tag="whT")
            nc.vector.tensor_copy(whT[:], pack[:, WHT_O:WHT_O + ND])
            u_sb = []
            for dc in range(ND):
                u_t = cpool.tile([P, D], BF16, name=f"u{dc}", tag=f"u{dc}")
                nc.vector.tensor_copy(u_t[:], pack[:, U_O + dc * D:U_O + (dc + 1) * D])
                u_sb.append(u_t)

            # ---- persistent SBUF tensors (all bf16) ----
            # one PSUM strip holds the 16 hs accumulators for the whole body
            hs_ps = ps_mm.tile([P, OC], F32, name="psmm", tag="psmm")

            headT = [ppool.tile([P, S], BF16, name=f"headT{dc}", tag=f"headT{dc}")
                     for dc in range(ND)]
            depT = [ppool.tile([P, S], BF16, name=f"depT{dc}", tag=f"depT{dc}")
                    for dc in range(ND)]
            atf = [ppool.tile([P, S], BF16, name=f"atf{eb}", tag=f"atf{eb}")
                   for eb in range(ND)]
            hs_colb = ppool.tile([P, NI], F32, name="hs_colb", tag="hs_colb")

            eng_ctr = [0]

            def load_group(src_dram, g):
                # [128, GB*D] bf16, cast from f32 in the DMA (gpsimd/SWDGE)
                nat = stage.tile([P, GB * D], BF16, name="nat", tag="nat")
                src = src_dram[g * GB * P:(g + 1) * GB * P, :]
                src3 = src.rearrange("(j p) d -> p j d", p=P)
                nc.gpsimd.dma_start(nat[:].rearrange("p (j d) -> p j d", d=D), src3)
                return nat

            def transpose_group(nat, dstT, g):
                # 8 PE transposes (bf16, 1cyc/row) -> [128,512] PSUM f32 ->
                # bf16 collect copies alternating DVE/ACT
                for dc in range(ND):
                    pst = ps_t.tile([P, GB * P], BF16, name="pst", tag="pst")
                    for j in range(GB):
                        nc.tensor.transpose(
                            pst[:, j * P:(j + 1) * P],
                            nat[:, j * D + dc * P: j * D + dc * P + P],
                            eye[:],
                        )
                    dst = dstT[dc][:, g * GB * P:(g + 1) * GB * P]
                    eng_ctr[0] += 1
                    if eng_ctr[0] % 2 == 0:
                        nc.vector.tensor_copy(dst, pst[:])
                    else:
                        nc.scalar.copy(dst, pst[:])

            def hs_group(g):
                # hs for this group's 4 row-blocks on the PE: per block,
                # 2 accumulating [128d x 128i]^T @ [128d x 1] matmuls with
                # whT as the moving vector -> hs_ps column; then one ACT
                # bias. ~100ns each, absorbed in PE's load-phase slack --
                # no DVE/Pool work at all.
                for ib in range(g * GB, (g + 1) * GB):
                    for dc in range(ND):
                        nc.tensor.matmul(
                            hs_ps[:, ib:ib + 1],
                            headT[dc][:, ib * P:(ib + 1) * P],
                            whT[:, dc:dc + 1],
                            start=(dc == 0),
                            stop=(dc == ND - 1),
                        )
                nc.scalar.activation(
                    hs_colb[:, g * GB:(g + 1) * GB],
                    hs_ps[:, g * GB:(g + 1) * GB], Ident, bias=b128
                )

            def atf_group(g):
                # ATf chunk ic=g from headT[:, g*512:(g+1)*512]
                for eb in range(ND):
                    pa = ps_mm.tile([P, OC], F32, name="psmm", tag="psmm")
                    for dc in range(ND):
                        nc.tensor.matmul(
                            pa[:],
                            u_sb[dc][:, eb * P:(eb + 1) * P],
                            headT[dc][:, g * OC:(g + 1) * OC],
                            start=(dc == 0),
                            stop=(dc == ND - 1),
                        )
                    # split the two bias/round chunks across ACT and DVE
                    if eb == 0:
                        nc.scalar.activation(
                            atf[eb][:, g * OC:(g + 1) * OC], pa[:], Ident,
                            bias=wdT[:, eb:eb + 1],
                        )
                    else:
                        nc.vector.tensor_scalar_add(
                            atf[eb][:, g * OC:(g + 1) * OC], pa[:],
                            wdT[:, eb:eb + 1],
                        )

            def out_pair(ib, oc):
                # two row-blocks x one col chunk -> one [128,1024] tile ->
                # ONE store to the contiguous [256,512] DRAM region (halves
                # the per-store dispatch cost that would otherwise pace the
                # train)
                ot = outbuf.tile([P, 2 * OC], BF16, name="ot", tag="ot")
                for j in range(2):
                    po = ps_mm.tile([P, OC], F32, name="psmm", tag="psmm")
                    for eb in range(ND):
                        nc.tensor.matmul(
                            po[:],
                            atf[eb][:, (ib + j) * P:(ib + j + 1) * P],
                            depT[eb][:, oc * OC:(oc + 1) * OC],
                            start=(eb == 0),
                            stop=(eb == ND - 1),
                        )
                    dst = ot[:, j * OC:(j + 1) * OC]
                    if (ib + j + oc) % 2 == 0:
                        nc.scalar.activation(
                            dst, po[:], Ident, bias=hs_colb[:, ib + j:ib + j + 1]
                        )
                    else:
                        nc.vector.tensor_scalar_add(
                            dst, po[:], hs_colb[:, ib + j:ib + j + 1]
                        )
                dram = out_d[ib * P:(ib + 2) * P, oc * OC:(oc + 1) * OC]
                nc.sync.dma_start(
                    dram.rearrange("(j p) d -> p j d", p=P),
                    ot[:].rearrange("p (j d) -> p j d", d=OC),
                )

            def body():
                # ---- loads in first-use order: col0 needs all heads plus
                # dep0; deps 1-3 are only touched from col1 (~t+15us) on ----
                nat_h, nat_p = {}, {}
                nat_h[0] = load_group(head_d, 0)
                nat_p[0] = load_group(dep_d, 0)
                for g in range(1, NG):
                    nat_h[g] = load_group(head_d, g)
                for g in range(1, NG):
                    nat_p[g] = load_group(dep_d, g)

                # ---- col-major sweep; head blocks + dep transposes drop in
                # as their loads land ----
                transpose_group(nat_h[0], headT, 0)
                atf_group(0)
                hs_group(0)
                transpose_group(nat_p[0], depT, 0)
                for ib in range(0, NI, 2):
                    k = ib // GB
                    if ib % GB == 0 and k > 0:
                        transpose_group(nat_h[k], headT, k)
                        atf_group(k)
                        hs_group(k)
                    out_pair(ib, 0)
                for oc in range(1, NO):
                    transpose_group(nat_p[oc], depT, oc)
                    for ib in range(0, NI, 2):
                        out_pair(ib, oc)

            nbody = int(unroll) if unroll else 1
            if reps > 1:
                with tc.For_i(0, reps, 1):
                    for _ in range(nbody):
                        body()
            else:
                for _ in range(nbody):
                    body()

    nc.finalize()
    return nc


_NC_CACHE = {}


def _get_nc(reps=1):
    if reps not in _NC_CACHE:
        _NC_CACHE[reps] = build_nc(reps)
    return _NC_CACHE[reps]


def make_in_maps(head, dep, edge_U, edge_W, edge_b):
    head = np.ascontiguousarray(np.asarray(head, dtype=np.float32))
    dep = np.ascontiguousarray(np.asarray(dep, dtype=np.float32))
    u = np.asarray(edge_U, dtype=np.float32)
    w = np.asarray(edge_W, dtype=np.float32).reshape(-1)
    wh, wd = w[:D], w[D:]
    pack = np.zeros((P, PACKW), np.float32)
    pack[:, EYE_O:EYE_O + P] = np.eye(P, dtype=np.float32)
    for dc in range(ND):
        pack[:, U_O + dc * D:U_O + (dc + 1) * D] = u[dc * P:(dc + 1) * P, :]
    pack[:, WHT_O:WHT_O + ND] = wh.reshape(ND, P).T
    pack[:, WDT_O:WDT_O + ND] = wd.reshape(ND, P).T
    pack[:, B_O] = float(np.asarray(edge_b).reshape(-1)[0])
    return [
        {"head": head[b], "dep": dep[b], "pack": pack}
        for b in range(B)
    ]


def kernel(head, dep, edge_U, edge_W, edge_b):
    nc = _get_nc()
    in_maps = make_in_maps(head, dep, edge_U, edge_W, edge_b)
    res = run_bass_kernel_spmd(nc, in_maps, core_ids=list(range(B)))
    return np.stack(
        [np.asarray(res.results[b]["out"]).astype(np.float32) for b in range(B)],
        axis=0,
    )
